# revision 1
# baseline (speedup 1.0000x reference)
"""Trainium2 Bass kernel for DrBCEncoder-style GNN message passing.

Strategy (8 NeuronCores, SPMD):
  - Nodes are dst-sharded: core c owns nodes [c*12500, (c+1)*12500), padded to
    12544 = 98*128 rows per core (total padded node space 100352 = 4*25088).
  - Each layer keeps the full node-major activation h [100352, 64] f32 in HBM
    (AllGather output); each core gathers rows h[src] for edges whose dst is in
    its shard via dma_gather (int16 indices -> 4 banks of 25088 rows).
  - segment-sum over edges is computed as a matmul: for each 128-edge chunk,
    PSUM[64 feat, 128 dst] += feats_chunk[128e, 64f].T @ onehot[128e, 128d]
    where onehot[e, d] = (dst_local[e] == d) * inv_deg[dst(e)] is built on the
    vector engines from an iota constant. This directly yields the transposed
    neighbor mean, which is the stationary operand for the neighbor-weight
    matmul.
  - Self path: h_tile [128, 64] is PE-transposed, then
    z[128n, 64f] = hT.T @ Ws_l.T + nmT.T @ Wn_l.T accumulated in PSUM.
  - LayerNorm over features runs on the free axis with per-partition stats,
    then relu + residual, store shard, AllGather for the next layer.

Host-side work is index preprocessing only (edge sort/bucketing, degree
bincount, layout packing, weight transposes).
"""
import sys

sys.path.insert(0, "/opt/trn_rl_repo")

import numpy as np

import concourse.bass as bass
import concourse.bacc as bacc
import concourse.tile as tile
from concourse import mybir
from concourse.bass_utils import run_bass_kernel_spmd

NCORES = 8
N_NODES = 100000
NODES_PER_CORE = 12500
PAD_PER_CORE = 12544            # 98 * 128
N_PAD = NCORES * PAD_PER_CORE   # 100352
TILES = PAD_PER_CORE // 128     # 98
BANKS = 4
BANK_ROWS = N_PAD // BANKS      # 25088 (< 32768 for int16 indices)
HID = 64
IN_DIM = 8
N_LAYERS = 3
LN_EPS = 1e-5

F32 = mybir.dt.float32
I16 = mybir.dt.int16
AOT = mybir.AluOpType
ACT_F = mybir.ActivationFunctionType

_program_cache = {}


def _remap(v):
    return (v // NODES_PER_CORE) * PAD_PER_CORE + (v % NODES_PER_CORE)


def _build_program(l_bank):
    """Build + compile the SPMD Bass program for a given per-bank slot size."""
    cb = l_bank // 128          # chunks per bank
    C = BANKS * cb              # chunks per tile
    lb16 = l_bank // 16

    nc = bacc.Bacc("TRN2", target_bir_lowering=False, debug=False,
                   num_devices=NCORES)

    idx_in = nc.dram_tensor("idx", [TILES, 128, BANKS * lb16], I16,
                            kind="ExternalInput")
    meta_in = nc.dram_tensor("meta", [TILES, 128, 2 * C], F32,
                             kind="ExternalInput")
    xt_in = nc.dram_tensor("xt", [IN_DIM, PAD_PER_CORE], F32,
                           kind="ExternalInput")
    w_in_t = nc.dram_tensor("w_in_t", [IN_DIM, HID], F32, kind="ExternalInput")
    ws_t = nc.dram_tensor("ws_t", [N_LAYERS, HID, HID], F32, kind="ExternalInput")
    wn_t = nc.dram_tensor("wn_t", [N_LAYERS, HID, HID], F32, kind="ExternalInput")
    bias_b = nc.dram_tensor("bias_b", [N_LAYERS, 128, HID], F32, kind="ExternalInput")
    gamma_b = nc.dram_tensor("gamma_b", [N_LAYERS, 128, HID], F32, kind="ExternalInput")
    beta_b = nc.dram_tensor("beta_b", [N_LAYERS, 128, HID], F32, kind="ExternalInput")
    b_in_b = nc.dram_tensor("b_in_b", [128, HID], F32, kind="ExternalInput")
    iota_in = nc.dram_tensor("iota", [128, 128], F32, kind="ExternalInput")
    ident_in = nc.dram_tensor("ident", [128, 128], F32, kind="ExternalInput")
    core_base_in = nc.dram_tensor("core_base", [1, 1], mybir.dt.int32,
                                  kind="ExternalInput")  # unused on device
    h_out = nc.dram_tensor("h_out", [PAD_PER_CORE, HID], F32,
                           kind="ExternalOutput")

    with tile.TileContext(nc) as tc:
        with (
            tc.tile_pool(name="const", bufs=1) as cp,
            tc.tile_pool(name="io", bufs=3) as iop,
            tc.tile_pool(name="feats", bufs=2) as fp,
            tc.tile_pool(name="oh", bufs=6) as ohp,
            tc.tile_pool(name="ln", bufs=3) as lnp,
            tc.tile_pool(name="ps_agg", bufs=2, space="PSUM") as ps_agg,
            tc.tile_pool(name="ps_tp", bufs=2, space="PSUM") as ps_tp,
            tc.tile_pool(name="ps_z", bufs=2, space="PSUM") as ps_z,
            tc.tile_pool(name="dram", bufs=1, space="DRAM") as dp,
        ):
            # ---- constants ----
            iota_t = cp.tile([128, 128], F32, tag="iota")
            nc.sync.dma_start(iota_t[:], iota_in[:])
            eps_t = cp.tile([128, 1], F32, tag="eps")
            nc.vector.memset(eps_t[:], LN_EPS)
            ident_t = cp.tile([128, 128], F32, tag="ident")
            nc.sync.dma_start(ident_t[:], ident_in[:])
            w_in_sb = cp.tile([IN_DIM, HID], F32, tag="w_in")
            nc.sync.dma_start(w_in_sb[:], w_in_t[:])
            b_in_sb = cp.tile([128, HID], F32, tag="b_in")
            nc.sync.dma_start(b_in_sb[:], b_in_b[:])
            ws_sb, wn_sb, bias_sb, gamma_sb, beta_sb = [], [], [], [], []
            for l in range(N_LAYERS):
                w1 = cp.tile([HID, HID], F32, tag=f"ws{l}")
                nc.sync.dma_start(w1[:], ws_t[l])
                ws_sb.append(w1)
                w2 = cp.tile([HID, HID], F32, tag=f"wn{l}")
                nc.sync.dma_start(w2[:], wn_t[l])
                wn_sb.append(w2)
                b1 = cp.tile([128, HID], F32, tag=f"bias{l}")
                nc.sync.dma_start(b1[:], bias_b[l])
                bias_sb.append(b1)
                g1 = cp.tile([128, HID], F32, tag=f"gamma{l}")
                nc.sync.dma_start(g1[:], gamma_b[l])
                gamma_sb.append(g1)
                be1 = cp.tile([128, HID], F32, tag=f"beta{l}")
                nc.sync.dma_start(be1[:], beta_b[l])
                beta_sb.append(be1)

            # ---- DRAM buffers ----
            h_bufs = [
                dp.tile([N_PAD, HID], F32, tag=f"h_buf{i}", name=f"h_buf{i}",
                        addr_space="Shared")
                for i in range(N_LAYERS)
            ]
            shards = [
                dp.tile([PAD_PER_CORE, HID], F32, tag=f"shard{i}",
                        name=f"shard{i}")
                for i in range(N_LAYERS)
            ]

            # ---- phase 0: h0 = relu(x @ W_in.T + b_in) for own shard ----
            for t in range(TILES):
                xt_sb = iop.tile([IN_DIM, 128], F32, tag="xt")
                nc.sync.dma_start(xt_sb[:], xt_in[:, t * 128:(t + 1) * 128])
                h0_ps = ps_z.tile([128, HID], F32, tag="z")
                nc.tensor.matmul(h0_ps[:], xt_sb[:], w_in_sb[:],
                                 start=True, stop=True)
                h0_sb = lnp.tile([128, HID], F32, tag="hnew")
                nc.vector.scalar_tensor_tensor(
                    h0_sb[:], h0_ps[:], 0.0, b_in_sb[:], AOT.bypass, AOT.add)
                h0r_sb = lnp.tile([128, HID], F32, tag="hnew2")
                nc.scalar.activation(h0r_sb[:], h0_sb[:], ACT_F.Relu)
                nc.sync.dma_start(shards[0][t * 128:(t + 1) * 128, :], h0r_sb[:])
            nc.gpsimd.collective_compute(
                "AllGather", AOT.bypass,
                ins=[shards[0].opt()], outs=[h_bufs[0].opt()],
                replica_groups=[list(range(NCORES))])

            # ---- layers ----
            for l in range(N_LAYERS):
                src_buf = h_bufs[l]
                own_shard = shards[l]
                for t in range(TILES):
                    idx_t = iop.tile([128, BANKS * lb16], I16, tag="idx")
                    nc.sync.dma_start(idx_t[:], idx_in[t])
                    meta_t = iop.tile([128, 2 * C], F32, tag="meta")
                    nc.sync.dma_start(meta_t[:], meta_in[t])

                    feats = fp.tile([128, C, HID], F32, tag="feats")
                    for b in range(BANKS):
                        nc.gpsimd.dma_gather(
                            feats[:, b * cb:(b + 1) * cb, :],
                            src_buf[b * BANK_ROWS:(b + 1) * BANK_ROWS, :],
                            idx_t[:, b * lb16:(b + 1) * lb16],
                            l_bank, l_bank, HID,
                            single_packet=(l_bank <= 1024))

                    agg = ps_agg.tile([HID, 128], F32, tag="agg")
                    for k in range(C):
                        oh = ohp.tile([128, 128], F32, tag="oh")
                        nc.any.tensor_scalar(
                            oh[:], iota_t[:],
                            meta_t[:, k:k + 1], meta_t[:, C + k:C + k + 1],
                            AOT.is_equal, AOT.mult)
                        nc.tensor.matmul(agg[:], feats[:, k, :], oh[:],
                                         start=(k == 0), stop=(k == C - 1))

                    nmT = lnp.tile([HID, 128], F32, tag="nmT")
                    nc.vector.tensor_copy(nmT[:], agg[:])

                    h_t = iop.tile([128, HID], F32, tag="h_t")
                    nc.sync.dma_start(
                        h_t[:], own_shard[t * 128:(t + 1) * 128, :])
                    tp_ps = ps_tp.tile([HID, 128], F32, tag="tp")
                    nc.tensor.transpose(tp_ps[:], h_t[:], ident_t[:])
                    hT_t = lnp.tile([HID, 128], F32, tag="hT")
                    nc.vector.tensor_copy(hT_t[:], tp_ps[:])

                    z_ps = ps_z.tile([128, HID], F32, tag="z")
                    nc.tensor.matmul(z_ps[:], hT_t[:], ws_sb[l][:],
                                     start=True, stop=False)
                    nc.tensor.matmul(z_ps[:], nmT[:], wn_sb[l][:],
                                     start=False, stop=True)

                    # LayerNorm + affine + relu + residual
                    stats = lnp.tile([128, 4], F32, tag="stats")
                    zb = lnp.tile([128, HID], F32, tag="zb")
                    nc.vector.scalar_tensor_tensor(
                        zb[:], z_ps[:], 0.0, bias_sb[l][:],
                        AOT.bypass, AOT.add, accum_out=stats[:, 0:1])
                    zsq = lnp.tile([128, HID], F32, tag="zsq")
                    nc.scalar.activation(zsq[:], zb[:], ACT_F.Square,
                                         accum_out=stats[:, 1:2])
                    mstat = lnp.tile([128, 2], F32, tag="mstat")
                    nc.vector.tensor_scalar(
                        mstat[:], stats[:, 0:2], 1.0 / HID, None, AOT.mult)
                    m2 = lnp.tile([128, 1], F32, tag="m2")
                    nc.vector.tensor_tensor(
                        m2[:], mstat[:, 0:1], mstat[:, 0:1], AOT.mult)
                    var = lnp.tile([128, 1], F32, tag="var")
                    nc.vector.tensor_tensor(
                        var[:], mstat[:, 1:2], m2[:], AOT.subtract)
                    std = lnp.tile([128, 1], F32, tag="std")
                    nc.scalar.activation(std[:], var[:], ACT_F.Sqrt,
                                         bias=eps_t[:])
                    rstd = lnp.tile([128, 1], F32, tag="rstd")
                    nc.vector.reciprocal(rstd[:], std[:])
                    t2 = lnp.tile([128, HID], F32, tag="t2")
                    nc.vector.tensor_scalar(
                        t2[:], zb[:], mstat[:, 0:1], rstd[:],
                        AOT.subtract, AOT.mult)
                    t3 = lnp.tile([128, HID], F32, tag="t3")
                    nc.vector.scalar_tensor_tensor(
                        t3[:], t2[:], 0.0, gamma_sb[l][:], AOT.bypass, AOT.mult)
                    t4 = lnp.tile([128, HID], F32, tag="t4")
                    nc.vector.scalar_tensor_tensor(
                        t4[:], t3[:], 0.0, beta_sb[l][:], AOT.bypass, AOT.add)
                    h_new = lnp.tile([128, HID], F32, tag="hnew")
                    nc.vector.scalar_tensor_tensor(
                        h_new[:], t4[:], 0.0, h_t[:], AOT.max, AOT.add)

                    if l == N_LAYERS - 1:
                        nc.sync.dma_start(
                            h_out[t * 128:(t + 1) * 128, :], h_new[:])
                    else:
                        nc.sync.dma_start(
                            shards[l + 1][t * 128:(t + 1) * 128, :], h_new[:])
                if l < N_LAYERS - 1:
                    nc.gpsimd.collective_compute(
                        "AllGather", AOT.bypass,
                        ins=[shards[l + 1].opt()],
                        outs=[h_bufs[l + 1].opt()],
                        replica_groups=[list(range(NCORES))])

    nc.compile()
    return nc


def _preprocess(x, edge_src, edge_dst, W_in, b_in, Ws_self, Ws_neigh,
                biases, gammas, betas):
    """Pure index/layout preprocessing on the host. Returns (in_maps, l_bank)."""
    src = edge_src.astype(np.int64)
    dst = edge_dst.astype(np.int64)
    rsrc = _remap(src)
    rdst = _remap(dst)

    tile_g = rdst // 128              # global tile id in padded space, 0..783
    dst_loc = (rdst % 128).astype(np.float32)
    bank = rsrc // BANK_ROWS
    idx_loc = (rsrc - bank * BANK_ROWS).astype(np.int16)

    deg = np.bincount(dst, minlength=N_NODES)
    invdeg = np.where(deg > 0, 1.0 / np.maximum(deg, 1), 0.0).astype(np.float32)
    inv_e = invdeg[dst]

    n_groups = NCORES * TILES * BANKS
    key = tile_g * BANKS + bank
    order = np.argsort(key, kind="stable")
    key_s = key[order]
    counts = np.bincount(key_s, minlength=n_groups)
    l_bank = max(128, int(np.ceil(counts.max() / 128)) * 128)
    cb = l_bank // 128
    C = BANKS * cb
    lb16 = l_bank // 16

    starts = np.zeros(n_groups, dtype=np.int64)
    starts[1:] = np.cumsum(counts)[:-1]
    rank = np.arange(len(src)) - starts[key_s]
    pos = key_s * l_bank + rank       # global padded position

    total = n_groups * l_bank
    idx_full = np.zeros(total, dtype=np.int16)
    idx_full[pos] = idx_loc[order]
    dstl_full = np.full(total, -1.0, dtype=np.float32)
    dstl_full[pos] = dst_loc[order]
    inv_full = np.zeros(total, dtype=np.float32)
    inv_full[pos] = inv_e[order]

    # idx: [784, BANKS, l_bank] -> wrap16 -> replicate to 128 partitions
    idx_w = idx_full.reshape(NCORES * TILES, BANKS, lb16, 16)
    idx_w = idx_w.transpose(0, 1, 3, 2)                    # [784, B, 16, lb16]
    idx_w = np.broadcast_to(idx_w[:, :, None, :, :],
                            (NCORES * TILES, BANKS, 8, 16, lb16))
    idx_w = idx_w.transpose(0, 2, 3, 1, 4).reshape(
        NCORES, TILES, 128, BANKS * lb16).copy()

    # meta: positions within a tile wrap mod 128 across all banks' chunks
    dstl_w = dstl_full.reshape(NCORES * TILES, C, 128).transpose(0, 2, 1)
    inv_w = inv_full.reshape(NCORES * TILES, C, 128).transpose(0, 2, 1)
    meta = np.concatenate([dstl_w, inv_w], axis=2).reshape(
        NCORES, TILES, 128, 2 * C).astype(np.float32).copy()

    # xT per core
    xp = np.zeros((N_PAD, IN_DIM), dtype=np.float32)
    xp[_remap(np.arange(N_NODES))] = x
    xp = xp.reshape(NCORES, PAD_PER_CORE, IN_DIM)

    w_in_t = np.ascontiguousarray(W_in.T.astype(np.float32))
    ws_t = np.ascontiguousarray(Ws_self.transpose(0, 2, 1).astype(np.float32))
    wn_t = np.ascontiguousarray(Ws_neigh.transpose(0, 2, 1).astype(np.float32))
    bias_b = np.ascontiguousarray(
        np.broadcast_to(biases[:, None, :], (N_LAYERS, 128, HID)).astype(np.float32))
    gamma_b = np.ascontiguousarray(
        np.broadcast_to(gammas[:, None, :], (N_LAYERS, 128, HID)).astype(np.float32))
    beta_b = np.ascontiguousarray(
        np.broadcast_to(betas[:, None, :], (N_LAYERS, 128, HID)).astype(np.float32))
    b_in_bc = np.ascontiguousarray(
        np.broadcast_to(b_in[None, :], (128, HID)).astype(np.float32))
    iota = np.tile(np.arange(128, dtype=np.float32), (128, 1))
    ident = np.eye(128, dtype=np.float32)

    in_maps = []
    for c in range(NCORES):
        in_maps.append({
            "idx": idx_w[c],
            "meta": meta[c],
            "xt": np.ascontiguousarray(xp[c].T),
            "w_in_t": w_in_t,
            "ws_t": ws_t,
            "wn_t": wn_t,
            "bias_b": bias_b,
            "gamma_b": gamma_b,
            "beta_b": beta_b,
            "b_in_b": b_in_bc,
            "iota": iota,
            "ident": ident,
            "core_base": np.array([[c]], dtype=np.int32),
        })
    return in_maps, l_bank


def kernel(**inputs):
    in_maps, l_bank = _preprocess(
        np.asarray(inputs["x"]), np.asarray(inputs["edge_src"]),
        np.asarray(inputs["edge_dst"]), np.asarray(inputs["W_in"]),
        np.asarray(inputs["b_in"]), np.asarray(inputs["Ws_self"]),
        np.asarray(inputs["Ws_neigh"]), np.asarray(inputs["biases"]),
        np.asarray(inputs["gammas"]), np.asarray(inputs["betas"]))

    if l_bank not in _program_cache:
        _program_cache[l_bank] = _build_program(l_bank)
    nc = _program_cache[l_bank]

    res = run_bass_kernel_spmd(nc, in_maps, list(range(NCORES)))
    out = np.concatenate(
        [res.results[c]["h_out"][:NODES_PER_CORE] for c in range(NCORES)],
        axis=0)
    return out.astype(np.float32)



# revision 14
# speedup vs baseline: 1.8391x; 1.8391x over previous
"""Trainium2 Bass kernel for DrBCEncoder-style GNN message passing.

Strategy (8 NeuronCores, SPMD, dst-sharded nodes):
  - Nodes dst-sharded: core c owns rows [c*12500, (c+1)*12500), padded to
    12544 = 98*128 rows (total padded node space 100352 = 4*25088 bank rows).
  - Activations kept in HBM two ways per layer:
      h_full  [100352, 128] bf16 (64 feats + 64 zero pad -> 256B rows, the
              dma_gather elem granularity), AllGather output, gather source.
      shard   [12544, 64] f32 local shard (residual source, exact).
  - Per 128-dst tile: gather h_full[src] rows for the tile's edges with
    nc.gpsimd.dma_gather. Edges are bucketed per (tile, src-bank) with the
    4 banks = 4 x 25088 rows (int16 index range). Each bank's gather call is
    issued on its own SWDGE queue (queue_num=bank): the 4 Q7 core pairs
    generate descriptors concurrently (~3.5x measured vs single queue).
    Pad slots use idx=-1 (trailing negatives are skipped by the ucode).
  - segment-sum as matmul: PSUM[64f, 128dst] += feats_k[128e, 64f].T @
    oh_k[128e, 128dst], oh built on DVE in bf16 (4x perf mode) via
    is_equal(iota, dst_loc) * inv_deg.
  - Self path: hT tile via HWDGE DMA-transpose of the bf16 padded shard;
    z[128n,64] = hT.T @ Ws' + nmT.T @ Wn' in bf16, accumulated in PSUM f32.
  - LayerNorm in f32 on the free axis, relu + residual (f32 shard), then
    store f32 shard + bf16 padded shard, AllGather for the next layer.

Host-side work is index preprocessing only (edge sort/bucketing, degree
bincount, layout packing, weight transposes, bf16 casts).
"""
import sys

sys.path.insert(0, "/opt/trn_rl_repo")

import ml_dtypes
import numpy as np

import concourse.bass as bass
import concourse.bacc as bacc
import concourse.tile as tile
from concourse import mybir
from concourse.bass_utils import run_bass_kernel_spmd

NCORES = 8
N_NODES = 100000
NODES_PER_CORE = 12500
PAD_PER_CORE = 12544            # 98 * 128
N_PAD = NCORES * PAD_PER_CORE   # 100352
TILES = PAD_PER_CORE // 128     # 98
BANKS = 4
BANK_ROWS = N_PAD // BANKS      # 25088 (< 32768 for int16 indices)
HID = 64
ROW = 2 * HID                   # padded bf16 row: 64 feats + 64 zeros = 256B
IN_DIM = 8
N_LAYERS = 3
LN_EPS = 1e-5

F32 = mybir.dt.float32
BF16 = mybir.dt.bfloat16
I16 = mybir.dt.int16
AOT = mybir.AluOpType
ACT_F = mybir.ActivationFunctionType
BF = ml_dtypes.bfloat16

_program_cache = {}

import os
DBG_NQ = int(os.environ.get("GNN_NQ", "4"))       # gather queues used
DBG_NEGPAD = os.environ.get("GNN_NEGPAD", "1") == "1"
DBG_TP = os.environ.get("GNN_TP", "1") == "1"      # hwdge dma transpose


def _remap(v):
    return (v // NODES_PER_CORE) * PAD_PER_CORE + (v % NODES_PER_CORE)


def _build_program(l_bank, affine_trivial):
    """SPMD Bass program. l_bank: padded edges per (tile, bank) slot.
    affine_trivial: gammas==1 and betas==0, skip the two affine ops."""
    cb = l_bank // 128          # chunks per bank
    C = BANKS * cb              # chunks per tile
    lb16 = l_bank // 16
    moff = BANKS * lb16         # int16 offset of meta within combined buffer

    nc = bacc.Bacc("TRN2", target_bir_lowering=False, debug=False,
                   num_devices=NCORES, num_swdge_queues=4,
                   dynamic_dma_scratch_size=65536)

    idx_in = nc.dram_tensor("idx", [TILES, 128, BANKS * lb16], I16,
                            kind="ExternalInput")
    meta_in = nc.dram_tensor("meta", [TILES, 128, 2 * C], F32,
                             kind="ExternalInput")
    xt_in = nc.dram_tensor("xt", [IN_DIM, PAD_PER_CORE], F32,
                           kind="ExternalInput")
    w_in_t = nc.dram_tensor("w_in_t", [IN_DIM, HID], F32, kind="ExternalInput")
    ws_t = nc.dram_tensor("ws_t", [N_LAYERS, HID, HID], BF16,
                          kind="ExternalInput")
    wn_t = nc.dram_tensor("wn_t", [N_LAYERS, HID, HID], BF16,
                          kind="ExternalInput")
    bias_b = nc.dram_tensor("bias_b", [N_LAYERS, 128, HID], F32,
                            kind="ExternalInput")
    gamma_b = nc.dram_tensor("gamma_b", [N_LAYERS, 128, HID], F32,
                             kind="ExternalInput")
    beta_b = nc.dram_tensor("beta_b", [N_LAYERS, 128, HID], F32,
                            kind="ExternalInput")
    b_in_b = nc.dram_tensor("b_in_b", [128, HID], F32, kind="ExternalInput")
    iota_in = nc.dram_tensor("iota", [128, 128], BF16, kind="ExternalInput")
    ident_in = nc.dram_tensor("ident", [128, 128], BF16, kind="ExternalInput")
    cnt_in = nc.dram_tensor("cnt", [1, TILES * BANKS], mybir.dt.int32,
                            kind="ExternalInput")
    h_out = nc.dram_tensor("h_out", [PAD_PER_CORE, HID], F32,
                           kind="ExternalOutput")

    with tile.TileContext(nc) as tc:
        with (
            tc.tile_pool(name="const", bufs=1) as cp,
            tc.tile_pool(name="io", bufs=4) as iop,
            tc.tile_pool(name="feats", bufs=6) as fp,
            tc.tile_pool(name="oh", bufs=8) as ohp,
            tc.tile_pool(name="ln", bufs=3) as lnp,
            tc.tile_pool(name="hb", bufs=3) as hbp,
            tc.tile_pool(name="ps_agg", bufs=2, space="PSUM") as ps_agg,
            tc.tile_pool(name="ps_z", bufs=2, space="PSUM") as ps_z,
            tc.tile_pool(name="dram", bufs=1, space="DRAM") as dp,
        ):
            # ---- constants ----
            iota_t = cp.tile([128, 128], BF16, tag="iota")
            nc.sync.dma_start(iota_t[:], iota_in[:])
            identb_t = None
            if not DBG_TP:
                identb_t = cp.tile([128, 128], BF16, tag="identb")
                nc.sync.dma_start(identb_t[:], ident_in[:])
            cnt_sb = cp.tile([1, TILES * BANKS], mybir.dt.int32, tag="cnt")
            nc.sync.dma_start(cnt_sb[:], cnt_in[:])
            eps_t = cp.tile([128, 1], F32, tag="eps")
            nc.vector.memset(eps_t[:], LN_EPS)
            w_in_sb = cp.tile([IN_DIM, HID], F32, tag="w_in")
            nc.sync.dma_start(w_in_sb[:], w_in_t[:])
            b_in_sb = cp.tile([128, HID], F32, tag="b_in")
            nc.sync.dma_start(b_in_sb[:], b_in_b[:])
            ws_sb, wn_sb, bias_sb, gamma_sb, beta_sb = [], [], [], [], []
            for l in range(N_LAYERS):
                w1 = cp.tile([HID, HID], BF16, tag=f"ws{l}")
                nc.sync.dma_start(w1[:], ws_t[l])
                ws_sb.append(w1)
                w2 = cp.tile([HID, HID], BF16, tag=f"wn{l}")
                nc.sync.dma_start(w2[:], wn_t[l])
                wn_sb.append(w2)
                b1 = cp.tile([128, HID], F32, tag=f"bias{l}")
                nc.sync.dma_start(b1[:], bias_b[l])
                bias_sb.append(b1)
                if not affine_trivial:
                    g1 = cp.tile([128, HID], F32, tag=f"gamma{l}")
                    nc.sync.dma_start(g1[:], gamma_b[l])
                    gamma_sb.append(g1)
                    be1 = cp.tile([128, HID], F32, tag=f"beta{l}")
                    nc.sync.dma_start(be1[:], beta_b[l])
                    beta_sb.append(be1)

            # ---- DRAM buffers ----
            h_bufs = [
                dp.tile([N_PAD, ROW], BF16, tag=f"h_buf{i}", name=f"h_buf{i}",
                        addr_space="Shared")
                for i in range(N_LAYERS)
            ]
            pads = [
                dp.tile([PAD_PER_CORE, ROW], BF16, tag=f"pad{i}",
                        name=f"pad{i}")
                for i in range(N_LAYERS)
            ]
            shards = [
                dp.tile([PAD_PER_CORE, HID], F32, tag=f"shard{i}",
                        name=f"shard{i}")
                for i in range(N_LAYERS)
            ]

            # zero the feats / hb pool buffers once: gather skips pad slots
            # (idx=-1) leaving stale bytes that must stay finite; hb keeps
            # its zero pad columns [64:128] forever.
            for _ in range(6):
                fz = fp.tile([128, C, ROW], BF16, tag="feats")
                nc.vector.memset(fz[:], 0.0)
            for _ in range(3):
                hz = hbp.tile([128, ROW], BF16, tag="hb")
                nc.gpsimd.memset(hz[:], 0.0)

            # ---- phase 0: h0 = relu(x @ W_in.T + b_in) for own shard ----
            for t in range(TILES):
                xt_sb = iop.tile([IN_DIM, 128], F32, tag="xt")
                nc.sync.dma_start(xt_sb[:], xt_in[:, t * 128:(t + 1) * 128])
                h0_ps = ps_z.tile([128, HID], F32, tag="z")
                nc.tensor.matmul(h0_ps[:], xt_sb[:], w_in_sb[:],
                                 start=True, stop=True)
                h0_sb = lnp.tile([128, HID], F32, tag="hnew")
                nc.vector.scalar_tensor_tensor(
                    h0_sb[:], h0_ps[:], 0.0, b_in_sb[:], AOT.bypass, AOT.add)
                h0r_sb = lnp.tile([128, HID], F32, tag="hnew2")
                nc.scalar.activation(h0r_sb[:], h0_sb[:], ACT_F.Relu)
                hb = hbp.tile([128, ROW], BF16, tag="hb")
                nc.scalar.copy(hb[:, 0:HID], h0r_sb[:])
                nc.sync.dma_start(shards[0][t * 128:(t + 1) * 128, :],
                                  h0r_sb[:])
                nc.sync.dma_start(pads[0][t * 128:(t + 1) * 128, :], hb[:])
            nc.gpsimd.collective_compute(
                "AllGather", AOT.bypass,
                ins=[pads[0].opt()], outs=[h_bufs[0].opt()],
                replica_groups=[list(range(NCORES))])

            # ---- layers ----
            # depth-4 register rotation per bank: the WAR dep between a
            # gather and the count reload for the same register otherwise
            # head-of-line-blocks the Pool sequencer and serializes queues.
            cnt_regs = [[nc.gpsimd.alloc_register(f"cnt{b}_{r}")
                         for b in range(BANKS)] for r in range(4)] \
                if DBG_NEGPAD else None
            for l in range(N_LAYERS):
                src_buf = h_bufs[l]
                last = l == N_LAYERS - 1
                for t in range(TILES):
                    im_t = iop.tile([128, BANKS * lb16], I16, tag="idx")
                    nc.sync.dma_start(im_t[:], idx_in[t])
                    meta_t = iop.tile([128, 2 * C], F32, tag="meta")
                    nc.sync.dma_start(meta_t[:], meta_in[t])
                    meta = meta_t

                    feats = fp.tile([128, C, ROW], BF16, tag="feats")
                    for b in range(BANKS):
                        if DBG_NEGPAD:
                            g = t * BANKS + b
                            nreg = cnt_regs[t % 4][b]
                            nc.gpsimd.reg_load(nreg,
                                               cnt_sb[0:1, g:g + 1])
                        else:
                            nreg = l_bank
                        nc.gpsimd.dma_gather(
                            feats[:, b * cb:(b + 1) * cb, :],
                            src_buf[b * BANK_ROWS:(b + 1) * BANK_ROWS, :],
                            im_t[:, b * lb16:(b + 1) * lb16],
                            l_bank, nreg, ROW,
                            single_packet=(l_bank <= 1024),
                            queue_num=b % DBG_NQ)

                    agg = ps_agg.tile([HID, 128], F32, tag="agg")
                    for k in range(C):
                        oh = ohp.tile([128, 128], BF16, tag="oh")
                        nc.vector.tensor_scalar(
                            oh[:], iota_t[:],
                            meta[:, k:k + 1], meta[:, C + k:C + k + 1],
                            AOT.is_equal, AOT.mult)
                        nc.tensor.matmul(agg[:], feats[:, k, 0:HID], oh[:],
                                         start=(k == 0), stop=(k == C - 1))

                    nmT = lnp.tile([HID, 128], BF16, tag="nmT")
                    nc.scalar.copy(nmT[:], agg[:])

                    h_t = iop.tile([128, HID], F32, tag="h_t")
                    nc.scalar.dma_start(
                        h_t[:], shards[l][t * 128:(t + 1) * 128, :])
                    if DBG_TP:
                        hTt = lnp.tile([128, 128], BF16, tag="hTt")
                        nc.sync.dma_start_transpose(
                            hTt[:], pads[l][t * 128:(t + 1) * 128, :])
                        hT64 = hTt[0:HID, :]
                    else:
                        hbt = lnp.tile([128, HID], BF16, tag="hbt")
                        nc.vector.tensor_copy(hbt[:], h_t[:])
                        tp_ps = ps_agg.tile([HID, 128], BF16, tag="tp")
                        nc.tensor.transpose(tp_ps[:], hbt[:], identb_t[:])
                        hTc = lnp.tile([HID, 128], BF16, tag="hTc")
                        nc.vector.tensor_copy(hTc[:], tp_ps[:])
                        hT64 = hTc[:]

                    z_ps = ps_z.tile([128, HID], F32, tag="z")
                    nc.tensor.matmul(z_ps[:], hT64, ws_sb[l][:],
                                     start=True, stop=False)
                    nc.tensor.matmul(z_ps[:], nmT[:], wn_sb[l][:],
                                     start=False, stop=True)

                    # LayerNorm + affine + relu + residual
                    stats = lnp.tile([128, 2], F32, tag="stats")
                    zb = lnp.tile([128, HID], F32, tag="zb")
                    nc.vector.scalar_tensor_tensor(
                        zb[:], z_ps[:], 0.0, bias_sb[l][:],
                        AOT.bypass, AOT.add, accum_out=stats[:, 0:1])
                    zsq = lnp.tile([128, HID], F32, tag="zsq")
                    nc.scalar.activation(zsq[:], zb[:], ACT_F.Square,
                                         accum_out=stats[:, 1:2])
                    mstat = lnp.tile([128, 2], F32, tag="mstat")
                    nc.vector.tensor_scalar(
                        mstat[:], stats[:, 0:2], 1.0 / HID, None, AOT.mult)
                    m2 = lnp.tile([128, 1], F32, tag="m2")
                    nc.vector.tensor_tensor(
                        m2[:], mstat[:, 0:1], mstat[:, 0:1], AOT.mult)
                    var = lnp.tile([128, 1], F32, tag="var")
                    nc.vector.tensor_tensor(
                        var[:], mstat[:, 1:2], m2[:], AOT.subtract)
                    std = lnp.tile([128, 1], F32, tag="std")
                    nc.scalar.activation(std[:], var[:], ACT_F.Sqrt,
                                         bias=eps_t[:])
                    rstd = lnp.tile([128, 1], F32, tag="rstd")
                    nc.vector.reciprocal(rstd[:], std[:])
                    t2 = lnp.tile([128, HID], F32, tag="t2")
                    nc.vector.tensor_scalar(
                        t2[:], zb[:], mstat[:, 0:1], rstd[:],
                        AOT.subtract, AOT.mult)
                    t4 = t2
                    if not affine_trivial:
                        t3 = lnp.tile([128, HID], F32, tag="t3")
                        nc.vector.scalar_tensor_tensor(
                            t3[:], t2[:], 0.0, gamma_sb[l][:],
                            AOT.bypass, AOT.mult)
                        t4 = lnp.tile([128, HID], F32, tag="t4")
                        nc.vector.scalar_tensor_tensor(
                            t4[:], t3[:], 0.0, beta_sb[l][:],
                            AOT.bypass, AOT.add)
                    h_new = lnp.tile([128, HID], F32, tag="hnew")
                    nc.vector.scalar_tensor_tensor(
                        h_new[:], t4[:], 0.0, h_t[:], AOT.max, AOT.add)

                    if last:
                        nc.sync.dma_start(
                            h_out[t * 128:(t + 1) * 128, :], h_new[:])
                    else:
                        hb = hbp.tile([128, ROW], BF16, tag="hb")
                        nc.scalar.copy(hb[:, 0:HID], h_new[:])
                        nc.sync.dma_start(
                            shards[l + 1][t * 128:(t + 1) * 128, :], h_new[:])
                        nc.sync.dma_start(
                            pads[l + 1][t * 128:(t + 1) * 128, :], hb[:])
                if not last:
                    nc.gpsimd.collective_compute(
                        "AllGather", AOT.bypass,
                        ins=[pads[l + 1].opt()],
                        outs=[h_bufs[l + 1].opt()],
                        replica_groups=[list(range(NCORES))])

    nc.compile()
    return nc


def _preprocess(x, edge_src, edge_dst, W_in, b_in, Ws_self, Ws_neigh,
                biases, gammas, betas):
    """Pure index/layout preprocessing on the host."""
    src = edge_src.astype(np.int64)
    dst = edge_dst.astype(np.int64)
    rsrc = _remap(src)
    rdst = _remap(dst)

    tile_g = rdst // 128              # global tile id in padded space, 0..783
    dst_loc = (rdst % 128).astype(np.float32)
    bank = rsrc // BANK_ROWS
    idx_loc = (rsrc - bank * BANK_ROWS).astype(np.int16)

    deg = np.bincount(dst, minlength=N_NODES)
    invdeg = np.where(deg > 0, 1.0 / np.maximum(deg, 1), 0.0).astype(np.float32)
    inv_e = invdeg[dst]

    n_groups = NCORES * TILES * BANKS
    key = tile_g * BANKS + bank
    order = np.argsort(key, kind="stable")
    key_s = key[order]
    counts = np.bincount(key_s, minlength=n_groups)
    l_bank = max(128, int(np.ceil(counts.max() / 128)) * 128)
    cb = l_bank // 128
    C = BANKS * cb
    lb16 = l_bank // 16

    starts = np.zeros(n_groups, dtype=np.int64)
    starts[1:] = np.cumsum(counts)[:-1]
    rank = np.arange(len(src)) - starts[key_s]
    pos = key_s * l_bank + rank       # global padded position

    total = n_groups * l_bank
    padval = -1 if DBG_NEGPAD else 0
    idx_full = np.full(total, padval, dtype=np.int16)  # pad: skipped by ucode
    idx_full[pos] = idx_loc[order]
    dstl_full = np.full(total, -1.0, dtype=np.float32)
    dstl_full[pos] = dst_loc[order]
    inv_full = np.zeros(total, dtype=np.float32)
    inv_full[pos] = inv_e[order]

    # idx: [784, BANKS, l_bank] -> wrap16 -> replicate to 128 partitions
    idx_w = idx_full.reshape(NCORES * TILES, BANKS, lb16, 16)
    idx_w = idx_w.transpose(0, 1, 3, 2)                    # [784, B, 16, lb16]
    idx_w = np.broadcast_to(idx_w[:, :, None, :, :],
                            (NCORES * TILES, BANKS, 8, 16, lb16))
    idx_w = idx_w.transpose(0, 2, 3, 1, 4).reshape(
        NCORES, TILES, 128, BANKS * lb16)

    # meta (bf16): positions within a tile wrap mod 128 across all chunks
    dstl_w = dstl_full.reshape(NCORES * TILES, C, 128).transpose(0, 2, 1)
    inv_w = inv_full.reshape(NCORES * TILES, C, 128).transpose(0, 2, 1)
    meta = np.concatenate([dstl_w, inv_w], axis=2).reshape(
        NCORES, TILES, 128, 2 * C).astype(np.float32)
    idx_w = np.ascontiguousarray(idx_w)
    meta = np.ascontiguousarray(meta)

    # xT per core
    xp = np.zeros((N_PAD, IN_DIM), dtype=np.float32)
    xp[_remap(np.arange(N_NODES))] = x
    xp = xp.reshape(NCORES, PAD_PER_CORE, IN_DIM)

    w_in_t = np.ascontiguousarray(W_in.T.astype(np.float32))
    ws_t = np.ascontiguousarray(
        Ws_self.transpose(0, 2, 1).astype(np.float32)).astype(BF)
    wn_t = np.ascontiguousarray(
        Ws_neigh.transpose(0, 2, 1).astype(np.float32)).astype(BF)
    bias_b = np.ascontiguousarray(
        np.broadcast_to(biases[:, None, :],
                        (N_LAYERS, 128, HID)).astype(np.float32))
    gamma_b = np.ascontiguousarray(
        np.broadcast_to(gammas[:, None, :],
                        (N_LAYERS, 128, HID)).astype(np.float32))
    beta_b = np.ascontiguousarray(
        np.broadcast_to(betas[:, None, :],
                        (N_LAYERS, 128, HID)).astype(np.float32))
    b_in_bc = np.ascontiguousarray(
        np.broadcast_to(b_in[None, :], (128, HID)).astype(np.float32))
    iota = np.tile(np.arange(128, dtype=np.float32), (128, 1)).astype(BF)
    ident = np.eye(128, dtype=np.float32).astype(BF)

    affine_trivial = bool(np.all(gammas == 1.0) and np.all(betas == 0.0))

    counts_pc = counts.reshape(NCORES, TILES, BANKS).astype(np.int32)

    in_maps = []
    for c in range(NCORES):
        in_maps.append({
            "idx": idx_w[c],
            "meta": meta[c],
            "xt": np.ascontiguousarray(xp[c].T),
            "w_in_t": w_in_t,
            "ws_t": ws_t,
            "wn_t": wn_t,
            "bias_b": bias_b,
            "gamma_b": gamma_b,
            "beta_b": beta_b,
            "b_in_b": b_in_bc,
            "iota": iota,
            "ident": ident,
            "cnt": np.ascontiguousarray(
                counts_pc[c].reshape(1, TILES * BANKS)),
        })
    return in_maps, l_bank, affine_trivial


def kernel(**inputs):
    in_maps, l_bank, affine_trivial = _preprocess(
        np.asarray(inputs["x"]), np.asarray(inputs["edge_src"]),
        np.asarray(inputs["edge_dst"]), np.asarray(inputs["W_in"]),
        np.asarray(inputs["b_in"]), np.asarray(inputs["Ws_self"]),
        np.asarray(inputs["Ws_neigh"]), np.asarray(inputs["biases"]),
        np.asarray(inputs["gammas"]), np.asarray(inputs["betas"]))

    key = (l_bank, affine_trivial, DBG_NQ, DBG_NEGPAD, DBG_TP)
    if key not in _program_cache:
        _program_cache[key] = _build_program(l_bank, affine_trivial)
    nc = _program_cache[key]

    res = run_bass_kernel_spmd(nc, in_maps, list(range(NCORES)))
    out = np.concatenate(
        [res.results[c]["h_out"][:NODES_PER_CORE] for c in range(NCORES)],
        axis=0)
    return out.astype(np.float32)


# revision 15
# speedup vs baseline: 1.9546x; 1.0628x over previous
"""Trainium2 Bass kernel for DrBCEncoder-style GNN message passing.

Strategy (8 NeuronCores, SPMD, dst-sharded nodes):
  - Nodes dst-sharded: core c owns rows [c*12500, (c+1)*12500), padded to
    12544 = 98*128 rows (total padded node space 100352 = 4*25088 bank rows).
  - Activations kept in HBM two ways per layer:
      h_full  [100352, 128] bf16 (64 feats + 64 zero pad -> 256B rows, the
              dma_gather elem granularity), AllGather output, gather source.
      shard   [12544, 64] f32 local shard (residual source, exact).
  - Per 128-dst tile: gather h_full[src] rows for the tile's edges with
    nc.gpsimd.dma_gather. Edges are bucketed per (tile, src-bank) with the
    4 banks = 4 x 25088 rows (int16 index range). Each bank's gather call is
    issued on its own SWDGE queue (queue_num=bank): the 4 Q7 core pairs
    generate descriptors concurrently (~3.5x measured vs single queue).
    Pad slots use idx=-1 (trailing negatives are skipped by the ucode).
  - segment-sum as matmul: PSUM[64f, 128dst] += feats_k[128e, 64f].T @
    oh_k[128e, 128dst], oh built on DVE in bf16 (4x perf mode) via
    is_equal(iota, dst_loc) * inv_deg.
  - Self path: hT tile via HWDGE DMA-transpose of the bf16 padded shard;
    z[128n,64] = hT.T @ Ws' + nmT.T @ Wn' in bf16, accumulated in PSUM f32.
  - LayerNorm in f32 on the free axis, relu + residual (f32 shard), then
    store f32 shard + bf16 padded shard, AllGather for the next layer.

Host-side work is index preprocessing only (edge sort/bucketing, degree
bincount, layout packing, weight transposes, bf16 casts).
"""
import sys

sys.path.insert(0, "/opt/trn_rl_repo")

import ml_dtypes
import numpy as np

import concourse.bass as bass
import concourse.bacc as bacc
import concourse.tile as tile
from concourse import mybir
from concourse.bass_utils import run_bass_kernel_spmd

NCORES = 8
N_NODES = 100000
NODES_PER_CORE = 12500
PAD_PER_CORE = 12544            # 98 * 128
N_PAD = NCORES * PAD_PER_CORE   # 100352
TILES = PAD_PER_CORE // 128     # 98
BANKS = 4
BANK_ROWS = N_PAD // BANKS      # 25088 (< 32768 for int16 indices)
HID = 64
ROW = 2 * HID                   # padded bf16 row: 64 feats + 64 zeros = 256B
IN_DIM = 8
N_LAYERS = 3
LN_EPS = 1e-5

F32 = mybir.dt.float32
BF16 = mybir.dt.bfloat16
I16 = mybir.dt.int16
AOT = mybir.AluOpType
ACT_F = mybir.ActivationFunctionType
BF = ml_dtypes.bfloat16

_program_cache = {}

import os
DBG_NQ = int(os.environ.get("GNN_NQ", "4"))       # gather queues used
DBG_NEGPAD = os.environ.get("GNN_NEGPAD", "1") == "1"
DBG_TP = os.environ.get("GNN_TP", "1") == "1"      # hwdge dma transpose
DBG_HOSTOH = os.environ.get("GNN_HOSTOH", "1") == "1"  # host-precomputed oh


def _remap(v):
    return (v // NODES_PER_CORE) * PAD_PER_CORE + (v % NODES_PER_CORE)


def _build_program(l_bank, affine_trivial):
    """SPMD Bass program. l_bank: padded edges per (tile, bank) slot.
    affine_trivial: gammas==1 and betas==0, skip the two affine ops."""
    cb = l_bank // 128          # chunks per bank
    C = BANKS * cb              # chunks per tile
    lb16 = l_bank // 16
    moff = BANKS * lb16         # int16 offset of meta within combined buffer

    nc = bacc.Bacc("TRN2", target_bir_lowering=False, debug=False,
                   num_devices=NCORES, num_swdge_queues=4,
                   dynamic_dma_scratch_size=65536)

    idx_in = nc.dram_tensor("idx", [TILES, 128, BANKS * lb16], I16,
                            kind="ExternalInput")
    meta_in = nc.dram_tensor("meta", [TILES, 128, 2 * C], F32,
                             kind="ExternalInput")
    xt_in = nc.dram_tensor("xt", [IN_DIM, PAD_PER_CORE], F32,
                           kind="ExternalInput")
    w_in_t = nc.dram_tensor("w_in_t", [IN_DIM, HID], F32, kind="ExternalInput")
    ws_t = nc.dram_tensor("ws_t", [N_LAYERS, HID, HID], BF16,
                          kind="ExternalInput")
    wn_t = nc.dram_tensor("wn_t", [N_LAYERS, HID, HID], BF16,
                          kind="ExternalInput")
    bias_b = nc.dram_tensor("bias_b", [N_LAYERS, 128, HID], F32,
                            kind="ExternalInput")
    gamma_b = nc.dram_tensor("gamma_b", [N_LAYERS, 128, HID], F32,
                             kind="ExternalInput")
    beta_b = nc.dram_tensor("beta_b", [N_LAYERS, 128, HID], F32,
                            kind="ExternalInput")
    b_in_b = nc.dram_tensor("b_in_b", [128, HID], F32, kind="ExternalInput")
    iota_in = nc.dram_tensor("iota", [128, 128], BF16, kind="ExternalInput")
    ident_in = nc.dram_tensor("ident", [128, 128], BF16, kind="ExternalInput")
    cnt_in = nc.dram_tensor("cnt", [1, TILES * BANKS], mybir.dt.int32,
                            kind="ExternalInput")
    oh_in = None
    if DBG_HOSTOH:
        oh_in = nc.dram_tensor("ohp", [TILES, 128, C * 128], BF16,
                               kind="ExternalInput")
    h_out = nc.dram_tensor("h_out", [PAD_PER_CORE, HID], F32,
                           kind="ExternalOutput")

    with tile.TileContext(nc) as tc:
        with (
            tc.tile_pool(name="const", bufs=1) as cp,
            tc.tile_pool(name="io", bufs=4) as iop,
            tc.tile_pool(name="feats", bufs=6) as fp,
            tc.tile_pool(name="oh", bufs=(3 if DBG_HOSTOH else 8)) as ohp,
            tc.tile_pool(name="ln", bufs=3) as lnp,
            tc.tile_pool(name="hb", bufs=3) as hbp,
            tc.tile_pool(name="ps_agg", bufs=2, space="PSUM") as ps_agg,
            tc.tile_pool(name="ps_z", bufs=2, space="PSUM") as ps_z,
            tc.tile_pool(name="dram", bufs=1, space="DRAM") as dp,
        ):
            # ---- constants ----
            iota_t = cp.tile([128, 128], BF16, tag="iota")
            nc.sync.dma_start(iota_t[:], iota_in[:])
            identb_t = None
            if not DBG_TP:
                identb_t = cp.tile([128, 128], BF16, tag="identb")
                nc.sync.dma_start(identb_t[:], ident_in[:])
            cnt_sb = cp.tile([1, TILES * BANKS], mybir.dt.int32, tag="cnt")
            nc.sync.dma_start(cnt_sb[:], cnt_in[:])
            eps_t = cp.tile([128, 1], F32, tag="eps")
            nc.vector.memset(eps_t[:], LN_EPS)
            w_in_sb = cp.tile([IN_DIM, HID], F32, tag="w_in")
            nc.sync.dma_start(w_in_sb[:], w_in_t[:])
            b_in_sb = cp.tile([128, HID], F32, tag="b_in")
            nc.sync.dma_start(b_in_sb[:], b_in_b[:])
            ws_sb, wn_sb, bias_sb, gamma_sb, beta_sb = [], [], [], [], []
            for l in range(N_LAYERS):
                w1 = cp.tile([HID, HID], BF16, tag=f"ws{l}")
                nc.sync.dma_start(w1[:], ws_t[l])
                ws_sb.append(w1)
                w2 = cp.tile([HID, HID], BF16, tag=f"wn{l}")
                nc.sync.dma_start(w2[:], wn_t[l])
                wn_sb.append(w2)
                b1 = cp.tile([128, HID], F32, tag=f"bias{l}")
                nc.sync.dma_start(b1[:], bias_b[l])
                bias_sb.append(b1)
                if not affine_trivial:
                    g1 = cp.tile([128, HID], F32, tag=f"gamma{l}")
                    nc.sync.dma_start(g1[:], gamma_b[l])
                    gamma_sb.append(g1)
                    be1 = cp.tile([128, HID], F32, tag=f"beta{l}")
                    nc.sync.dma_start(be1[:], beta_b[l])
                    beta_sb.append(be1)

            # ---- DRAM buffers ----
            h_bufs = [
                dp.tile([N_PAD, ROW], BF16, tag=f"h_buf{i}", name=f"h_buf{i}",
                        addr_space="Shared")
                for i in range(N_LAYERS)
            ]
            pads = [
                dp.tile([PAD_PER_CORE, ROW], BF16, tag=f"pad{i}",
                        name=f"pad{i}")
                for i in range(N_LAYERS)
            ]
            shards = [
                dp.tile([PAD_PER_CORE, HID], F32, tag=f"shard{i}",
                        name=f"shard{i}")
                for i in range(N_LAYERS)
            ]

            # zero the feats / hb pool buffers once: gather skips pad slots
            # (idx=-1) leaving stale bytes that must stay finite; hb keeps
            # its zero pad columns [64:128] forever.
            for _ in range(6):
                fz = fp.tile([128, C, ROW], BF16, tag="feats")
                nc.vector.memset(fz[:], 0.0)
            for _ in range(3):
                hz = hbp.tile([128, ROW], BF16, tag="hb")
                nc.gpsimd.memset(hz[:], 0.0)

            # ---- phase 0: h0 = relu(x @ W_in.T + b_in) for own shard ----
            for t in range(TILES):
                xt_sb = iop.tile([IN_DIM, 128], F32, tag="xt")
                nc.sync.dma_start(xt_sb[:], xt_in[:, t * 128:(t + 1) * 128])
                h0_ps = ps_z.tile([128, HID], F32, tag="z")
                nc.tensor.matmul(h0_ps[:], xt_sb[:], w_in_sb[:],
                                 start=True, stop=True)
                h0_sb = lnp.tile([128, HID], F32, tag="hnew")
                nc.vector.scalar_tensor_tensor(
                    h0_sb[:], h0_ps[:], 0.0, b_in_sb[:], AOT.bypass, AOT.add)
                h0r_sb = lnp.tile([128, HID], F32, tag="hnew2")
                nc.scalar.activation(h0r_sb[:], h0_sb[:], ACT_F.Relu)
                hb = hbp.tile([128, ROW], BF16, tag="hb")
                nc.scalar.copy(hb[:, 0:HID], h0r_sb[:])
                nc.sync.dma_start(shards[0][t * 128:(t + 1) * 128, :],
                                  h0r_sb[:])
                nc.sync.dma_start(pads[0][t * 128:(t + 1) * 128, :], hb[:])
            nc.gpsimd.collective_compute(
                "AllGather", AOT.bypass,
                ins=[pads[0].opt()], outs=[h_bufs[0].opt()],
                replica_groups=[list(range(NCORES))])

            # ---- layers ----
            # depth-4 register rotation per bank: the WAR dep between a
            # gather and the count reload for the same register otherwise
            # head-of-line-blocks the Pool sequencer and serializes queues.
            cnt_regs = [[nc.gpsimd.alloc_register(f"cnt{b}_{r}")
                         for b in range(BANKS)] for r in range(4)] \
                if DBG_NEGPAD else None
            for l in range(N_LAYERS):
                src_buf = h_bufs[l]
                last = l == N_LAYERS - 1
                for t in range(TILES):
                    im_t = iop.tile([128, BANKS * lb16], I16, tag="idx")
                    nc.sync.dma_start(im_t[:], idx_in[t])
                    if not DBG_HOSTOH:
                        meta_t = iop.tile([128, 2 * C], F32, tag="meta")
                        nc.sync.dma_start(meta_t[:], meta_in[t])
                        meta = meta_t

                    feats = fp.tile([128, C, ROW], BF16, tag="feats")
                    for b in range(BANKS):
                        if DBG_NEGPAD:
                            g = t * BANKS + b
                            nreg = cnt_regs[t % 4][b]
                            nc.gpsimd.reg_load(nreg,
                                               cnt_sb[0:1, g:g + 1])
                        else:
                            nreg = l_bank
                        nc.gpsimd.dma_gather(
                            feats[:, b * cb:(b + 1) * cb, :],
                            src_buf[b * BANK_ROWS:(b + 1) * BANK_ROWS, :],
                            im_t[:, b * lb16:(b + 1) * lb16],
                            l_bank, nreg, ROW,
                            single_packet=(l_bank <= 1024),
                            queue_num=b % DBG_NQ)

                    agg = ps_agg.tile([HID, 128], F32, tag="agg")
                    if DBG_HOSTOH:
                        oh_t = ohp.tile([128, C * 128], BF16, tag="oh")
                        nc.sync.dma_start(oh_t[:], oh_in[t])
                        for k in range(C):
                            nc.tensor.matmul(agg[:], feats[:, k, 0:HID],
                                             oh_t[:, k * 128:(k + 1) * 128],
                                             start=(k == 0), stop=(k == C - 1))
                    else:
                        for k in range(C):
                            oh = ohp.tile([128, 128], BF16, tag="oh")
                            nc.vector.tensor_scalar(
                                oh[:], iota_t[:],
                                meta[:, k:k + 1], meta[:, C + k:C + k + 1],
                                AOT.is_equal, AOT.mult)
                            nc.tensor.matmul(agg[:], feats[:, k, 0:HID],
                                             oh[:],
                                             start=(k == 0),
                                             stop=(k == C - 1))

                    nmT = lnp.tile([HID, 128], BF16, tag="nmT")
                    nc.scalar.copy(nmT[:], agg[:])

                    h_t = iop.tile([128, HID], F32, tag="h_t")
                    nc.scalar.dma_start(
                        h_t[:], shards[l][t * 128:(t + 1) * 128, :])
                    if DBG_TP:
                        hTt = lnp.tile([128, 128], BF16, tag="hTt")
                        nc.sync.dma_start_transpose(
                            hTt[:], pads[l][t * 128:(t + 1) * 128, :])
                        hT64 = hTt[0:HID, :]
                    else:
                        hbt = lnp.tile([128, HID], BF16, tag="hbt")
                        nc.vector.tensor_copy(hbt[:], h_t[:])
                        tp_ps = ps_agg.tile([HID, 128], BF16, tag="tp")
                        nc.tensor.transpose(tp_ps[:], hbt[:], identb_t[:])
                        hTc = lnp.tile([HID, 128], BF16, tag="hTc")
                        nc.vector.tensor_copy(hTc[:], tp_ps[:])
                        hT64 = hTc[:]

                    z_ps = ps_z.tile([128, HID], F32, tag="z")
                    nc.tensor.matmul(z_ps[:], hT64, ws_sb[l][:],
                                     start=True, stop=False)
                    nc.tensor.matmul(z_ps[:], nmT[:], wn_sb[l][:],
                                     start=False, stop=True)

                    # LayerNorm + affine + relu + residual
                    stats = lnp.tile([128, 2], F32, tag="stats")
                    zb = lnp.tile([128, HID], F32, tag="zb")
                    nc.vector.scalar_tensor_tensor(
                        zb[:], z_ps[:], 0.0, bias_sb[l][:],
                        AOT.bypass, AOT.add, accum_out=stats[:, 0:1])
                    zsq = lnp.tile([128, HID], F32, tag="zsq")
                    nc.scalar.activation(zsq[:], zb[:], ACT_F.Square,
                                         accum_out=stats[:, 1:2])
                    mstat = lnp.tile([128, 2], F32, tag="mstat")
                    nc.vector.tensor_scalar(
                        mstat[:], stats[:, 0:2], 1.0 / HID, None, AOT.mult)
                    m2 = lnp.tile([128, 1], F32, tag="m2")
                    nc.vector.tensor_tensor(
                        m2[:], mstat[:, 0:1], mstat[:, 0:1], AOT.mult)
                    var = lnp.tile([128, 1], F32, tag="var")
                    nc.vector.tensor_tensor(
                        var[:], mstat[:, 1:2], m2[:], AOT.subtract)
                    std = lnp.tile([128, 1], F32, tag="std")
                    nc.scalar.activation(std[:], var[:], ACT_F.Sqrt,
                                         bias=eps_t[:])
                    rstd = lnp.tile([128, 1], F32, tag="rstd")
                    nc.vector.reciprocal(rstd[:], std[:])
                    t2 = lnp.tile([128, HID], F32, tag="t2")
                    nc.vector.tensor_scalar(
                        t2[:], zb[:], mstat[:, 0:1], rstd[:],
                        AOT.subtract, AOT.mult)
                    t4 = t2
                    if not affine_trivial:
                        t3 = lnp.tile([128, HID], F32, tag="t3")
                        nc.vector.scalar_tensor_tensor(
                            t3[:], t2[:], 0.0, gamma_sb[l][:],
                            AOT.bypass, AOT.mult)
                        t4 = lnp.tile([128, HID], F32, tag="t4")
                        nc.vector.scalar_tensor_tensor(
                            t4[:], t3[:], 0.0, beta_sb[l][:],
                            AOT.bypass, AOT.add)
                    h_new = lnp.tile([128, HID], F32, tag="hnew")
                    nc.vector.scalar_tensor_tensor(
                        h_new[:], t4[:], 0.0, h_t[:], AOT.max, AOT.add)

                    if last:
                        nc.sync.dma_start(
                            h_out[t * 128:(t + 1) * 128, :], h_new[:])
                    else:
                        hb = hbp.tile([128, ROW], BF16, tag="hb")
                        nc.scalar.copy(hb[:, 0:HID], h_new[:])
                        nc.sync.dma_start(
                            shards[l + 1][t * 128:(t + 1) * 128, :], h_new[:])
                        nc.sync.dma_start(
                            pads[l + 1][t * 128:(t + 1) * 128, :], hb[:])
                if not last:
                    nc.gpsimd.collective_compute(
                        "AllGather", AOT.bypass,
                        ins=[pads[l + 1].opt()],
                        outs=[h_bufs[l + 1].opt()],
                        replica_groups=[list(range(NCORES))])

    nc.compile()
    return nc


def _preprocess(x, edge_src, edge_dst, W_in, b_in, Ws_self, Ws_neigh,
                biases, gammas, betas):
    """Pure index/layout preprocessing on the host."""
    src = edge_src.astype(np.int64)
    dst = edge_dst.astype(np.int64)
    rsrc = _remap(src)
    rdst = _remap(dst)

    tile_g = rdst // 128              # global tile id in padded space, 0..783
    dst_loc = (rdst % 128).astype(np.float32)
    bank = rsrc // BANK_ROWS
    idx_loc = (rsrc - bank * BANK_ROWS).astype(np.int16)

    deg = np.bincount(dst, minlength=N_NODES)
    invdeg = np.where(deg > 0, 1.0 / np.maximum(deg, 1), 0.0).astype(np.float32)
    inv_e = invdeg[dst]

    n_groups = NCORES * TILES * BANKS
    key = tile_g * BANKS + bank
    order = np.argsort(key, kind="stable")
    key_s = key[order]
    counts = np.bincount(key_s, minlength=n_groups)
    l_bank = max(128, int(np.ceil(counts.max() / 128)) * 128)
    cb = l_bank // 128
    C = BANKS * cb
    lb16 = l_bank // 16

    starts = np.zeros(n_groups, dtype=np.int64)
    starts[1:] = np.cumsum(counts)[:-1]
    rank = np.arange(len(src)) - starts[key_s]
    pos = key_s * l_bank + rank       # global padded position

    total = n_groups * l_bank
    padval = -1 if DBG_NEGPAD else 0
    idx_full = np.full(total, padval, dtype=np.int16)  # pad: skipped by ucode
    idx_full[pos] = idx_loc[order]
    dstl_full = np.full(total, -1.0, dtype=np.float32)
    dstl_full[pos] = dst_loc[order]
    inv_full = np.zeros(total, dtype=np.float32)
    inv_full[pos] = inv_e[order]

    # idx: [784, BANKS, l_bank] -> wrap16 -> replicate to 128 partitions
    idx_w = idx_full.reshape(NCORES * TILES, BANKS, lb16, 16)
    idx_w = idx_w.transpose(0, 1, 3, 2)                    # [784, B, 16, lb16]
    idx_w = np.broadcast_to(idx_w[:, :, None, :, :],
                            (NCORES * TILES, BANKS, 8, 16, lb16))
    idx_w = idx_w.transpose(0, 2, 3, 1, 4).reshape(
        NCORES, TILES, 128, BANKS * lb16)

    # meta: positions within a tile wrap mod 128 across all chunks
    dstl_w = dstl_full.reshape(NCORES * TILES, C, 128).transpose(0, 2, 1)
    inv_w = inv_full.reshape(NCORES * TILES, C, 128).transpose(0, 2, 1)
    meta = np.concatenate([dstl_w, inv_w], axis=2).reshape(
        NCORES, TILES, 128, 2 * C).astype(np.float32)
    idx_w = np.ascontiguousarray(idx_w)
    meta = np.ascontiguousarray(meta)

    oh_host = None
    if DBG_HOSTOH:
        # one-hot tiles, edge-major partitions: oh[tile_g, e_loc, k, dstloc]
        bank_e = key_s % BANKS
        rank_e = rank  # within (tile, bank) group, aligned with `order`
        tile_e = key_s // BANKS
        k_e = bank_e * cb + rank_e // 128        # chunk within tile
        e_loc = rank_e % 128                      # partition within chunk
        flat = ((tile_e * 128 + e_loc) * C + k_e) * 128 + rdst[order] % 128
        oh_host = np.zeros(NCORES * TILES * 128 * C * 128, dtype=np.uint16)
        vals = inv_e[order].astype(BF).view(np.uint16)
        oh_host[flat] = vals
        oh_host = oh_host.view(BF).reshape(NCORES, TILES, 128, C * 128)

    # xT per core
    xp = np.zeros((N_PAD, IN_DIM), dtype=np.float32)
    xp[_remap(np.arange(N_NODES))] = x
    xp = xp.reshape(NCORES, PAD_PER_CORE, IN_DIM)

    w_in_t = np.ascontiguousarray(W_in.T.astype(np.float32))
    ws_t = np.ascontiguousarray(
        Ws_self.transpose(0, 2, 1).astype(np.float32)).astype(BF)
    wn_t = np.ascontiguousarray(
        Ws_neigh.transpose(0, 2, 1).astype(np.float32)).astype(BF)
    bias_b = np.ascontiguousarray(
        np.broadcast_to(biases[:, None, :],
                        (N_LAYERS, 128, HID)).astype(np.float32))
    gamma_b = np.ascontiguousarray(
        np.broadcast_to(gammas[:, None, :],
                        (N_LAYERS, 128, HID)).astype(np.float32))
    beta_b = np.ascontiguousarray(
        np.broadcast_to(betas[:, None, :],
                        (N_LAYERS, 128, HID)).astype(np.float32))
    b_in_bc = np.ascontiguousarray(
        np.broadcast_to(b_in[None, :], (128, HID)).astype(np.float32))
    iota = np.tile(np.arange(128, dtype=np.float32), (128, 1)).astype(BF)
    ident = np.eye(128, dtype=np.float32).astype(BF)

    affine_trivial = bool(np.all(gammas == 1.0) and np.all(betas == 0.0))

    counts_pc = counts.reshape(NCORES, TILES, BANKS).astype(np.int32)

    in_maps = []
    for c in range(NCORES):
        in_maps.append({
            "idx": idx_w[c],
            "meta": meta[c],
            "xt": np.ascontiguousarray(xp[c].T),
            "w_in_t": w_in_t,
            "ws_t": ws_t,
            "wn_t": wn_t,
            "bias_b": bias_b,
            "gamma_b": gamma_b,
            "beta_b": beta_b,
            "b_in_b": b_in_bc,
            "iota": iota,
            "ident": ident,
            "cnt": np.ascontiguousarray(
                counts_pc[c].reshape(1, TILES * BANKS)),
            **({"ohp": oh_host[c]} if DBG_HOSTOH else {}),
        })
    return in_maps, l_bank, affine_trivial


def kernel(**inputs):
    in_maps, l_bank, affine_trivial = _preprocess(
        np.asarray(inputs["x"]), np.asarray(inputs["edge_src"]),
        np.asarray(inputs["edge_dst"]), np.asarray(inputs["W_in"]),
        np.asarray(inputs["b_in"]), np.asarray(inputs["Ws_self"]),
        np.asarray(inputs["Ws_neigh"]), np.asarray(inputs["biases"]),
        np.asarray(inputs["gammas"]), np.asarray(inputs["betas"]))

    key = (l_bank, affine_trivial, DBG_NQ, DBG_NEGPAD, DBG_TP)
    if key not in _program_cache:
        _program_cache[key] = _build_program(l_bank, affine_trivial)
    nc = _program_cache[key]

    res = run_bass_kernel_spmd(nc, in_maps, list(range(NCORES)))
    out = np.concatenate(
        [res.results[c]["h_out"][:NODES_PER_CORE] for c in range(NCORES)],
        axis=0)
    return out.astype(np.float32)
